# revision 42
# baseline (speedup 1.0000x reference)
"""Trainium2 Bass kernel for nn_MiMoV2Flash (7-layer MoE transformer fwd).

Sharding (8 NeuronCores):
  - tokens (B*S = 2048) sequence-parallel: core c owns tokens [c*256, (c+1)*256)
    (batch 0 -> cores 0-3, batch 1 -> cores 4-7)
  - attention: AllGather of per-core K/V each layer; per-core masks encode
    causal/sliding-window structure
  - MoE: expert-parallel (2 experts/core); AllGather of normed hidden,
    dense dispatch weighted by top-2 combine, ReduceScatter of partial outputs
  - lm_head: vocab-sharded (4000 vocab rows/core), AllGather of final hidden
Matmuls run as float32r (full-rate), residual stream kept in fp32.
"""
import sys

if "/opt/trn_rl_repo" not in sys.path:
    sys.path.insert(0, "/opt/trn_rl_repo")

import numpy as np

import concourse.bass as bass
import concourse.bacc as bacc
import concourse.mybir as mybir
from concourse import tile
from concourse import bass_utils
from concourse.masks import make_identity

# ---- model constants (hardcoded per problem spec) ----
NCORES = 8
DIM = 512
NH = 8
NKV = 2
HD = 64
E = 16
EPC = E // NCORES  # experts per core
EH = 512
V = 32000
VS = V // NCORES  # vocab slice per core
B = 2
S = 1024
N = B * S            # 2048 tokens
T = N // NCORES      # 256 tokens per core
P = 128
DT = DIM // P        # 4 feature tiles
ROPE_DIM = 32
FP16_MIN = float(np.finfo(np.float16).min)
EPS = 1e-6
LAYER_SPECS = [("dense", 10000), ("moe", 64), ("moe", 64), ("moe", 10000),
               ("moe", 64), ("moe", 64), ("moe", 10000)]

F32 = mybir.dt.float32
F32R = mybir.dt.float32r
I32 = mybir.dt.int32
ADD = mybir.AluOpType.add
SUB = mybir.AluOpType.subtract
MUL = mybir.AluOpType.mult
ISEQ = mybir.AluOpType.is_equal
AX = mybir.AxisListType.X
AF = mybir.ActivationFunctionType

KV_ELEMS = P * T + T * P          # k_fm [128,256] + v_tm [256,128] = 65536
H_ELEMS = DIM * T                 # 131072


def _mm_acc(nc, out_ap, pairs):
    n = len(pairs)
    for i, (l, r) in enumerate(pairs):
        nc.tensor.matmul(out_ap, l, r, start=(i == 0), stop=(i == n - 1))


def build_nc():
    nc = bacc.Bacc("TRN2", target_bir_lowering=False, debug=False,
                   num_devices=NCORES)
    RG = [list(range(NCORES))]

    # ---------------- DRAM I/O ----------------
    d_ids = nc.dram_tensor("ids", [P, 2], I32, kind="ExternalInput")
    d_emb = nc.dram_tensor("emb", [V, DIM], F32, kind="ExternalInput")
    d_ropec = nc.dram_tensor("ropec", [P, 32], F32, kind="ExternalInput")
    d_ropes = nc.dram_tensor("ropes", [P, 32], F32, kind="ExternalInput")
    d_maskg = nc.dram_tensor("maskg", [S, T], F32, kind="ExternalInput")
    d_masks = nc.dram_tensor("masks", [S, T], F32, kind="ExternalInput")
    d_embTv = nc.dram_tensor("embTv", [DIM, VS], F32R, kind="ExternalInput")
    d_normf = nc.dram_tensor("normf", [P, DT], F32, kind="ExternalInput")
    dL = []
    for l, (kind, _w) in enumerate(LAYER_SPECS):
        t = {}
        t["n1"] = nc.dram_tensor(f"L{l}_n1", [P, DT], F32, kind="ExternalInput")
        t["n2"] = nc.dram_tensor(f"L{l}_n2", [P, DT], F32, kind="ExternalInput")
        t["qw"] = nc.dram_tensor(f"L{l}_qw", [DIM, DIM], F32R, kind="ExternalInput")
        t["qb"] = nc.dram_tensor(f"L{l}_qb", [P, DT], F32, kind="ExternalInput")  # pre-scaled by 1/8
        t["kw"] = nc.dram_tensor(f"L{l}_kw", [DIM, P], F32R, kind="ExternalInput")
        t["kb"] = nc.dram_tensor(f"L{l}_kb", [P, 1], F32, kind="ExternalInput")
        t["vw"] = nc.dram_tensor(f"L{l}_vw", [DIM, P], F32R, kind="ExternalInput")
        t["vb"] = nc.dram_tensor(f"L{l}_vb", [P, 1], F32, kind="ExternalInput")
        t["ow"] = nc.dram_tensor(f"L{l}_ow", [DIM, DIM], F32R, kind="ExternalInput")
        t["ob"] = nc.dram_tensor(f"L{l}_ob", [P, DT], F32, kind="ExternalInput")
        t["sink"] = nc.dram_tensor(f"L{l}_sink", [1, 1], F32, kind="ExternalInput")
        if kind == "dense":
            t["w1"] = nc.dram_tensor(f"L{l}_w1", [DIM, 2048], F32R, kind="ExternalInput")
            t["b1"] = nc.dram_tensor(f"L{l}_b1", [P, 16], F32, kind="ExternalInput")
            t["w2"] = nc.dram_tensor(f"L{l}_w2", [2048, DIM], F32R, kind="ExternalInput")
            t["b2"] = nc.dram_tensor(f"L{l}_b2", [P, DT], F32, kind="ExternalInput")
        else:
            t["rw"] = nc.dram_tensor(f"L{l}_rw", [DIM, E], F32R, kind="ExternalInput")  # /0.1 folded
            t["rb"] = nc.dram_tensor(f"L{l}_rb", [1, E], F32R, kind="ExternalInput")
            t["W1"] = nc.dram_tensor(f"L{l}_W1", [EPC * DIM, EH], F32R, kind="ExternalInput")
            t["B1"] = nc.dram_tensor(f"L{l}_B1", [P, EPC * 4], F32, kind="ExternalInput")
            t["W2"] = nc.dram_tensor(f"L{l}_W2", [EPC * EH, DIM], F32R, kind="ExternalInput")
            t["B2"] = nc.dram_tensor(f"L{l}_B2", [P, EPC * 4], F32, kind="ExternalInput")
        dL.append(t)

    d_logits = nc.dram_tensor("logits", [N, VS], F32, kind="ExternalOutput")
    d_aux = nc.dram_tensor("aux", [1, 1], F32, kind="ExternalOutput")

    with tile.TileContext(nc) as tc:
        pid = nc.partition_id()
        b4 = (pid // 4) * 4  # first core of my batch

        # ---------------- persistent pools ----------------
        with (
            tc.tile_pool(name="const", bufs=1) as cpool,
            tc.tile_pool(name="xres", bufs=12) as xpool,
            tc.tile_pool(name="dram_loc", bufs=2, space="DRAM") as dloc,
            tc.tile_pool(name="dram_sh", bufs=2, space="DRAM") as dsh,
        ):
            ones32c = cpool.tile([P, 1], F32, name="ones32c")
            nc.gpsimd.memset(ones32c[:], 1.0)
            ones32r = cpool.tile([1, P], F32, name="ones32r")
            nc.gpsimd.memset(ones32r[:], 1.0)
            ones_c = cpool.tile([P, 1], F32R, name="ones_c")
            nc.vector.tensor_copy(ones_c[:], ones32c[:])
            ones_r = cpool.tile([1, P], F32R, name="ones_r")
            nc.vector.tensor_copy(ones_r[:], ones32r[:])
            ones32w = cpool.tile([P, 8], F32, name="ones32w")
            nc.gpsimd.memset(ones32w[:], 1.0)
            ones8r = cpool.tile([P, 8], F32R, name="ones8r")
            nc.vector.tensor_copy(ones8r[:], ones32w[:])
            id32 = cpool.tile([P, P], F32, name="id32")
            make_identity(nc, id32[:])
            id32r = cpool.tile([P, P], F32R, name="id32r")
            nc.vector.tensor_copy(id32r[:], id32[:])
            maskg_sb = cpool.tile([P, 8 * T], F32, name="maskg_sb")
            masks_sb = cpool.tile([P, 8 * T], F32, name="masks_sb")
            for kt in range(8):
                nc.sync.dma_start(maskg_sb[:, kt * T:(kt + 1) * T],
                                  d_maskg[kt * P:(kt + 1) * P, :])
                nc.sync.dma_start(masks_sb[:, kt * T:(kt + 1) * T],
                                  d_masks[kt * P:(kt + 1) * P, :])
            aux_acc = cpool.tile([1, 1], F32, name="aux_acc")
            nc.vector.memset(aux_acc[:], 0.0)
            eps_c = cpool.tile([1, 1], F32, name="eps_c")
            nc.scalar.mul(eps_c[:], ones32c[0:1, :], EPS)

            # ---------------- embedding + rope ----------------
            x = [None] * DT  # feature-major residual [128, 256] fp32, row=dt*128+p
            with (
                tc.tile_pool(name="emb_sb", bufs=2) as epool,
                tc.tile_pool(name="emb_ps", bufs=4, space="PSUM") as epps,
            ):
                ids_sb = epool.tile([P, 2], I32, name="ids_sb")
                nc.sync.dma_start(ids_sb[:], d_ids[:])
                g_t = []
                for t in range(2):
                    g = epool.tile([P, DIM], F32, name=f"g{t}", tag="g")
                    nc.gpsimd.indirect_dma_start(
                        out=g[:], out_offset=None, in_=d_emb[:],
                        in_offset=bass.IndirectOffsetOnAxis(ap=ids_sb[:, t:t + 1], axis=0),
                    )
                    g_t.append(g)
                ropec_sb = epool.tile([P, 32], F32, name="ropec_sb")
                ropes_sb = epool.tile([P, 32], F32, name="ropes_sb")
                nc.sync.dma_start(ropec_sb[:], d_ropec[:])
                nc.sync.dma_start(ropes_sb[:], d_ropes[:])
                for t in range(2):
                    g = g_t[t]
                    ct = ropec_sb[:, t * 16:(t + 1) * 16]
                    st = ropes_sb[:, t * 16:(t + 1) * 16]
                    xe = g[:, 0:ROPE_DIM:2]
                    xo = g[:, 1:ROPE_DIM:2]
                    ta = epool.tile([P, 16], F32, name="ta", tag="ta")
                    tb = epool.tile([P, 16], F32, name="tb", tag="tb")
                    ne = epool.tile([P, 16], F32, name="ne", tag="ne")
                    no = epool.tile([P, 16], F32, name="no", tag="no")
                    nc.vector.tensor_tensor(ta[:], xe, ct, MUL)
                    nc.vector.tensor_tensor(tb[:], xo, st, MUL)
                    nc.vector.tensor_tensor(ne[:], ta[:], tb[:], SUB)
                    nc.vector.tensor_tensor(ta[:], xo, ct, MUL)
                    nc.vector.tensor_tensor(tb[:], xe, st, MUL)
                    nc.vector.tensor_tensor(no[:], ta[:], tb[:], ADD)
                    nc.vector.tensor_copy(xe, ne[:])
                    nc.vector.tensor_copy(xo, no[:])
                # transpose token-major [128tok, 512] -> feature-major x[dt][:, t*128]
                for dt in range(DT):
                    x[dt] = xpool.tile([P, T], F32, name=f"x{dt}", tag="x")
                for t in range(2):
                    for dt in range(DT):
                        tp = epps.tile([P, P], F32, name="tp", tag="tp")
                        nc.tensor.transpose(tp[:], g_t[t][:, dt * P:(dt + 1) * P], id32[:])
                        nc.vector.tensor_copy(x[dt][:, t * P:(t + 1) * P], tp[:])

            # ---------------- helpers ----------------
            def rmsnorm(xt, w_dram, pool, psp, name, pstag="gen"):
                """fm rmsnorm: returns 4 f32r tiles [128,256]."""
                w_sb = pool.tile([P, DT], F32, name=f"{name}_w", tag=f"{name}_w")
                nc.sync.dma_start(w_sb[:], w_dram[:])
                sq = []
                for dt in range(DT):
                    s = pool.tile([P, T], F32R, name=f"{name}_sq{dt}",
                                  tag=f"{name}_sq", bufs=4)
                    nc.vector.tensor_tensor(s[:], xt[dt][:], xt[dt][:], MUL)
                    sq.append(s)
                ss = psp.tile([1, T], F32, name=f"{name}_ss", tag=pstag, bufs=1)
                _mm_acc(nc, ss[:], [(ones_c[:], s[:]) for s in sq])
                srow = pool.tile([1, T], F32, name=f"{name}_sr", tag=f"{name}_sr")
                nc.scalar.activation(srow[:], ss[:], AF.Sqrt, bias=eps_c[0:1, 0:1],
                                     scale=1.0 / DIM)
                rrow = pool.tile([1, T], F32R, name=f"{name}_rr", tag=f"{name}_rr")
                with nc.allow_low_precision(reason="f32r feeds full-rate matmul"):
                    nc.vector.reciprocal(rrow[:], srow[:])
                sc = psp.tile([P, T], F32, name=f"{name}_sc", tag=pstag, bufs=1)
                nc.tensor.matmul(sc[:], ones_r[:], rrow[:], start=True, stop=True)
                out = []
                for dt in range(DT):
                    o = pool.tile([P, T], F32R, name=f"{name}_o{dt}",
                                  tag=f"{name}_o", bufs=4)
                    nc.vector.scalar_tensor_tensor(
                        out=o[:], in0=xt[dt][:], scalar=w_sb[:, dt:dt + 1], in1=sc[:],
                        op0=MUL, op1=MUL)
                    out.append(o)
                return out

            def load_w(pool, dram, rows, cols, name, tag=None):
                """Load [rows, cols] dram weight into sbuf [128, (rows//128)*cols],
                col-block kt holds dram rows kt*128..+128."""
                kt_n = rows // P
                w = pool.tile([P, kt_n * cols], F32R, name=name, tag=tag or name)
                for kt in range(kt_n):
                    nc.sync.dma_start(w[:, kt * cols:(kt + 1) * cols],
                                      dram[kt * P:(kt + 1) * P, :])
                return w

            # ---------------- transformer layers ----------------
            for l, (kind, window) in enumerate(LAYER_SPECS):
                t = dL[l]
                mask_sb = maskg_sb if window > S else masks_sb

                # ===== attention =====
                with (
                    tc.tile_pool(name=f"at{l}", bufs=2) as ap,
                    tc.tile_pool(name=f"at{l}_e", bufs=3) as ep,
                    tc.tile_pool(name=f"at{l}_ps", bufs=2, space="PSUM") as aps,
                    tc.tile_pool(name=f"at{l}_ops", bufs=2, space="PSUM") as ops_,
                ):
                    xh = rmsnorm(x, t["n1"], ap, aps, f"n1_{l}")
                    qw = load_w(ap, t["qw"], DIM, DIM, f"qw{l}", tag="qw")
                    kw = load_w(ap, t["kw"], DIM, P, f"kw{l}", tag="kw")
                    vw = load_w(ap, t["vw"], DIM, P, f"vw{l}", tag="vw")
                    qb = ap.tile([P, DT], F32, name=f"qb{l}", tag="qb")
                    kb = ap.tile([P, 1], F32, name=f"kb{l}", tag="kb")
                    vb = ap.tile([P, 1], F32, name=f"vb{l}", tag="vb")
                    sink = ap.tile([1, 1], F32, name=f"sink{l}", tag="sink")
                    nc.sync.dma_start(qb[:], t["qb"][:])
                    nc.sync.dma_start(kb[:], t["kb"][:])
                    nc.sync.dma_start(vb[:], t["vb"][:])
                    nc.sync.dma_start(sink[:], t["sink"][:])

                    # q projection (scaled by 1/8; qb pre-scaled on host)
                    q_sb = []
                    for m in range(DT):
                        ps = aps.tile([P, T], F32, name=f"qp{m}", tag="qkv_ps")
                        _mm_acc(nc, ps[:], [(qw[:, kt * DIM + m * P: kt * DIM + (m + 1) * P],
                                             xh[kt][:]) for kt in range(DT)])
                        q = ap.tile([P, T], F32R, name=f"q{m}", tag="q", bufs=4)
                        nc.scalar.activation(q[:], ps[:], AF.Identity,
                                             bias=qb[:, m:m + 1], scale=0.125)
                        q_sb.append(q)
                    # k, v projections
                    kps = aps.tile([P, T], F32, name="kps", tag="qkv_ps")
                    _mm_acc(nc, kps[:], [(kw[:, kt * P:(kt + 1) * P], xh[kt][:])
                                         for kt in range(DT)])
                    k_sb = ap.tile([P, T], F32R, name="k_sb", tag="k_sb")
                    nc.scalar.activation(k_sb[:], kps[:], AF.Identity, bias=kb[:, 0:1])
                    vps = aps.tile([P, T], F32, name="vps", tag="qkv_ps")
                    _mm_acc(nc, vps[:], [(vw[:, kt * P:(kt + 1) * P], xh[kt][:])
                                         for kt in range(DT)])
                    v_sb = ap.tile([P, T], F32R, name="v_sb", tag="v_sb")
                    nc.scalar.activation(v_sb[:], vps[:], AF.Identity, bias=vb[:, 0:1])
                    # transpose v -> token-major [256, 128] (2 ptiles)
                    vt_sb = ap.tile([P, T], F32R, name="vt_sb", tag="vt_sb")
                    for th in range(2):
                        tp = aps.tile([P, P], F32R, name="vtp", tag="gen", bufs=1)
                        nc.tensor.transpose(tp[:], v_sb[:, th * P:(th + 1) * P], id32r[:])
                        nc.vector.tensor_copy(vt_sb[:, th * P:(th + 1) * P], tp[:])

                    # kv allgather
                    kv_in = dloc.tile([1, KV_ELEMS], F32R, name="kv_in", tag="kv_in")
                    nc.sync.dma_start(
                        kv_in[0:1, 0:P * T].rearrange("a (p f) -> (a p) f", p=P), k_sb[:])
                    for th in range(2):
                        nc.sync.dma_start(
                            kv_in[0:1, P * T + th * P * P: P * T + (th + 1) * P * P]
                            .rearrange("a (p f) -> (a p) f", p=P),
                            vt_sb[:, th * P:(th + 1) * P])
                    kv_ag = dsh.tile([NCORES, KV_ELEMS], F32R, addr_space="Shared",
                                     name="kv_ag", tag="kv_ag")
                    nc.gpsimd.collective_compute(
                        "AllGather", mybir.AluOpType.bypass, replica_groups=RG,
                        ins=[kv_in[:]], outs=[kv_ag[:]])

                    # load this batch's K (fm) and V (tm)
                    ksb = ap.tile([P, 4 * T], F32R, name="ksb", tag="ksb")
                    vsb = ap.tile([P, 8 * P], F32R, name="vsb", tag="vsb")
                    for j in range(4):
                        src = kv_ag[bass.ds(b4 + j, 1), 0:P * T]
                        nc.sync.dma_start(
                            ksb[:, j * T:(j + 1) * T],
                            src.rearrange("a (p f) -> (a p) f", p=P))
                        for th in range(2):
                            srcv = kv_ag[bass.ds(b4 + j, 1),
                                         P * T + th * P * P: P * T + (th + 1) * P * P]
                            nc.sync.dma_start(
                                vsb[:, (j * 2 + th) * P:(j * 2 + th + 1) * P],
                                srcv.rearrange("a (p f) -> (a p) f", p=P))

                    # scores^T / softmax / AV  (key-major).
                    # Head layout is permuted (host-side): q tile m holds head m
                    # on partitions 0-63 and head m+4 on partitions 64-127, so
                    # each head's q shares a base partition with its kv head.
                    # Per-head AV accumulators sit at psum base 0 (v3 requires
                    # matmul dst partition offset 0).
                    esink = ap.tile([1, 1], F32, name="esink", tag="esink")
                    nc.scalar.activation(esink[:], sink[:], AF.Exp)
                    o_heads = []
                    for h in range(NH):
                        mt = h % 4
                        pb = (h // 4) * HD
                        q_ap = q_sb[mt][pb:pb + HD, :]
                        o_ps = ops_.tile([HD, T], F32, name=f"oph{h}", tag="o_ps")
                        den = aps.tile([1, T], F32, name=f"den{h}", tag="den", bufs=1)
                        for kt in range(8):
                            sc = aps.tile([P, T], F32, name=f"sc{kt}", tag="sc")
                            nc.tensor.matmul(
                                sc[:],
                                ksb[pb:pb + HD,
                                    (kt // 2) * T + (kt % 2) * P:
                                    (kt // 2) * T + (kt % 2 + 1) * P],
                                q_ap, start=True, stop=True)
                            ei = ep.tile([P, T], F32, name="ei", tag="ei")
                            nc.vector.tensor_tensor(
                                ei[:], sc[:], mask_sb[:, kt * T:(kt + 1) * T], ADD)
                            e_sb = ep.tile([P, T], F32R, name="e_sb", tag="e_sb")
                            nc.scalar.activation(e_sb[:], ei[:], AF.Exp)
                            nc.tensor.matmul(den[:], ones_c[:], e_sb[:],
                                             start=(kt == 0), stop=(kt == 7))
                            nc.tensor.matmul(
                                o_ps[:],
                                vsb[:, kt * P + pb: kt * P + pb + HD],
                                e_sb[:], start=(kt == 0), stop=(kt == 7))
                        denf = ap.tile([1, T], F32, name=f"denf{h}", tag="denf")
                        nc.vector.tensor_scalar(
                            out=denf[:], in0=den[:],
                            scalar1=esink[0:1, 0:1], scalar2=None, op0=ADD)
                        rec = ap.tile([1, T], F32R, name=f"rec{h}", tag="rec")
                        with nc.allow_low_precision(reason="f32r feeds matmul"):
                            nc.vector.reciprocal(rec[:], denf[:])
                        scb = aps.tile([HD, T], F32, name=f"scb{h}", tag="gen", bufs=1)
                        nc.tensor.matmul(scb[:], ones_r[0:1, 0:HD], rec[:],
                                         start=True, stop=True)
                        bc = ap.tile([HD, T], F32R, name=f"bc{h}", tag="bc")
                        nc.scalar.copy(bc[:], scb[:])
                        o_h = ap.tile([HD, T], F32R, name=f"oh{h}", tag="o_sb",
                                      bufs=8)
                        nc.vector.tensor_tensor(o_h[:], o_ps[:], bc[:], MUL)
                        o_heads.append(o_h)
                    # o-projection: K=64 per head, ow loaded head-major [64, 8*512]
                    ow = ap.tile([HD, NH * DIM], F32R, name=f"ow{l}", tag="ow")
                    for h in range(NH):
                        nc.sync.dma_start(ow[:, h * DIM:(h + 1) * DIM],
                                          t["ow"][h * HD:(h + 1) * HD, :])
                    ob = ap.tile([P, DT], F32, name=f"ob{l}", tag="ob")
                    nc.sync.dma_start(ob[:], t["ob"][:])
                    xn = []
                    for dt in range(DT):
                        ps = aps.tile([P, T], F32, name=f"aop{dt}", tag="qkv_ps")
                        _mm_acc(nc, ps[:], [
                            (ow[:, h * DIM + dt * P: h * DIM + (dt + 1) * P],
                             o_heads[h][:]) for h in range(NH)])
                        nx = xpool.tile([P, T], F32, name=f"x{l}a{dt}", tag="x")
                        nc.vector.scalar_tensor_tensor(
                            out=nx[:], in0=ps[:], scalar=ob[:, dt:dt + 1], in1=x[dt][:],
                            op0=ADD, op1=ADD)
                        xn.append(nx)
                    x = xn

                # ===== FFN =====
                if kind == "dense":
                    with (
                        tc.tile_pool(name=f"ff{l}", bufs=1) as fp,
                        tc.tile_pool(name=f"ff{l}_b", bufs=2) as fpb,
                        tc.tile_pool(name=f"ff{l}_ps", bufs=2, space="PSUM") as fps,
                    ):
                        hh = rmsnorm(x, t["n2"], fpb, fps, f"n2_{l}")
                        w1 = load_w(fp, t["w1"], DIM, 2048, "w1d")
                        w2 = load_w(fp, t["w2"], 2048, DIM, "w2d")
                        b1 = fpb.tile([P, 16], F32, name="b1d")
                        b2 = fpb.tile([P, DT], F32, name="b2d")
                        nc.sync.dma_start(b1[:], t["b1"][:])
                        nc.sync.dma_start(b2[:], t["b2"][:])
                        g = []
                        for m in range(16):
                            ps = fps.tile([P, T], F32, name=f"hp{m}", tag="hp", bufs=3)
                            _mm_acc(nc, ps[:], [
                                (w1[:, kt * 2048 + m * P: kt * 2048 + (m + 1) * P],
                                 hh[kt][:]) for kt in range(DT)])
                            gm = fpb.tile([P, T], F32R, name=f"g{m}", tag="g",
                                          bufs=16)
                            nc.scalar.activation(gm[:], ps[:], AF.Silu,
                                                 bias=b1[:, m:m + 1])
                            g.append(gm)
                        xn = []
                        for dt in range(DT):
                            ps = fps.tile([P, T], F32, name=f"yp{dt}", tag="yp")
                            _mm_acc(nc, ps[:], [
                                (w2[:, m * DIM + dt * P: m * DIM + (dt + 1) * P],
                                 g[m][:]) for m in range(16)])
                            nx = xpool.tile([P, T], F32, name=f"x{l}f{dt}", tag="x")
                            nc.vector.scalar_tensor_tensor(
                                out=nx[:], in0=ps[:], scalar=b2[:, dt:dt + 1],
                                in1=x[dt][:], op0=ADD, op1=ADD)
                            xn.append(nx)
                        x = xn
                else:
                    # ===== MoE =====
                    with (
                        tc.tile_pool(name=f"mo{l}", bufs=1) as mp,
                        tc.tile_pool(name=f"mo{l}_b", bufs=2) as mpb,
                        tc.tile_pool(name=f"mo{l}_h", bufs=8) as mph,
                        tc.tile_pool(name=f"mo{l}_ps", bufs=2, space="PSUM") as mps,
                    ):
                        hh = rmsnorm(x, t["n2"], mpb, mps, f"n2_{l}")
                        h_in = dloc.tile([1, H_ELEMS], F32R, name="h_in", tag="h_in")
                        for dt in range(DT):
                            nc.sync.dma_start(
                                h_in[0:1, dt * P * T:(dt + 1) * P * T]
                                .rearrange("a (p f) -> (a p) f", p=P), hh[dt][:])
                        h_ag = dsh.tile([NCORES, H_ELEMS], F32R, addr_space="Shared",
                                        name="h_ag", tag="h_ag")
                        nc.gpsimd.collective_compute(
                            "AllGather", mybir.AluOpType.bypass, replica_groups=RG,
                            ins=[h_in[:]], outs=[h_ag[:]])

                        w1 = [load_w(mp, t["W1"][j * DIM:(j + 1) * DIM, :], DIM, EH,
                                     f"w1e{j}") for j in range(EPC)]
                        w2 = [load_w(mp, t["W2"][j * EH:(j + 1) * EH, :], EH, DIM,
                                     f"w2e{j}") for j in range(EPC)]
                        rw = load_w(mpb, t["rw"], DIM, E, "rw", tag="rw")
                        rb = mpb.tile([1, E], F32R, name="rb", tag="rb")
                        B1 = mpb.tile([P, EPC * 4], F32, name="B1", tag="B1")
                        B2 = mpb.tile([P, EPC * 4], F32, name="B2", tag="B2")
                        nc.sync.dma_start(rb[:], t["rb"][:])
                        nc.sync.dma_start(B1[:], t["B1"][:])
                        nc.sync.dma_start(B2[:], t["B2"][:])

                        cmb = [mpb.tile([1, N], F32R, name=f"cmb{j}", tag=f"cmb{j}",
                                        bufs=1) for j in range(EPC)]
                        S_ps = mps.tile([E, 8], F32, name="S_ps", tag="S_ps", bufs=1)
                        rs_in = dloc.tile([NCORES, H_ELEMS], F32, name="rs_in",
                                          tag="rs_in")
                        for r in range(NCORES):
                            hc = []
                            for kt in range(DT):
                                hcx = mph.tile([P, T], F32R, name=f"hc{kt}", tag="hc")
                                nc.sync.dma_start(
                                    hcx[:],
                                    h_ag[r:r + 1, kt * P * T:(kt + 1) * P * T]
                                    .rearrange("a (p f) -> (a p) f", p=P))
                                hc.append(hcx)
                            # router for 2 token-halves
                            for th in range(2):
                                lg_ps = mps.tile([P, E], F32, name="lg_ps", tag="lg_ps")
                                for kt in range(DT):
                                    nc.tensor.matmul(
                                        lg_ps[:], hc[kt][:, th * P:(th + 1) * P],
                                        rw[:, kt * E:(kt + 1) * E],
                                        start=(kt == 0), stop=False)
                                nc.tensor.matmul(lg_ps[:], ones_r[:], rb[:],
                                                 start=False, stop=True)
                                lg = mpb.tile([P, E], F32, name="lg", tag="lg")
                                nc.vector.tensor_copy(lg[:], lg_ps[:])
                                m8 = mpb.tile([P, 8], F32, name="m8", tag="m8")
                                nc.vector.max(m8[:], lg[:])
                                dd = mpb.tile([P, 1], F32, name="dd", tag="dd")
                                nc.vector.tensor_tensor(dd[:], m8[:, 1:2], m8[:, 0:1], SUB)
                                w2s = mpb.tile([P, 1], F32, name="w2s", tag="w2s")
                                nc.scalar.activation(w2s[:], dd[:], AF.Sigmoid)
                                w1s = mpb.tile([P, 1], F32, name="w1s", tag="w1s")
                                nc.vector.tensor_scalar(out=w1s[:], in0=w2s[:],
                                                        scalar1=-1.0, scalar2=1.0,
                                                        op0=MUL, op1=ADD)
                                cmb_tm = mpb.tile([P, EPC], F32, name="cmb_tm",
                                                  tag="cmb_tm")
                                for j in range(EPC):
                                    ecol = pid * EPC + j
                                    lcol = lg[:, bass.ds(ecol, 1)]
                                    eq1 = mpb.tile([P, 1], F32, name="eq1", tag="eq1")
                                    eq2 = mpb.tile([P, 1], F32, name="eq2", tag="eq2")
                                    nc.vector.tensor_tensor(eq1[:], lcol, m8[:, 0:1], ISEQ)
                                    nc.vector.tensor_tensor(eq2[:], lcol, m8[:, 1:2], ISEQ)
                                    t1 = mpb.tile([P, 1], F32, name="t1", tag="t1")
                                    nc.vector.tensor_scalar(
                                        out=t1[:], in0=eq1[:], scalar1=w1s[:, 0:1],
                                        scalar2=None, op0=MUL)
                                    # cmb[:, j] = eq2*w2 + eq1*w1
                                    nc.vector.scalar_tensor_tensor(
                                        out=cmb_tm[:, j:j + 1], in0=eq2[:],
                                        scalar=w2s[:, 0:1], in1=t1[:], op0=MUL, op1=ADD)
                                # transpose cmb_tm columns -> [1, 128] rows
                                for j in range(EPC):
                                    ct = mps.tile([1, P], F32, name="ct", tag="gen",
                                                  bufs=1)
                                    nc.tensor.transpose(ct[:], cmb_tm[:, j:j + 1],
                                                        id32[:])
                                    nc.vector.tensor_copy(
                                        cmb[j][0:1, r * T + th * P:
                                               r * T + (th + 1) * P], ct[:])
                                # aux: probs
                                e16 = mpb.tile([P, E], F32R, name="e16", tag="e16")
                                nc.scalar.activation(e16[:], lg[:], AF.Exp)
                                srow = mpb.tile([P, 1], F32, name="esum", tag="esum")
                                nc.vector.tensor_reduce(srow[:], e16[:], AX, ADD)
                                rr = mpb.tile([P, 1], F32, name="rr", tag="rr")
                                nc.vector.reciprocal(rr[:], srow[:])
                                probs = mpb.tile([P, E], F32R, name="probs", tag="probs")
                                nc.vector.tensor_scalar(out=probs[:], in0=e16[:],
                                                        scalar1=rr[:, 0:1], scalar2=None,
                                                        op0=MUL)
                                nc.tensor.matmul(S_ps[:], probs[:], ones8r[:],
                                                 start=(r == 0 and th == 0),
                                                 stop=(r == NCORES - 1 and th == 1))
                            # experts (dense dispatch)
                            yacc = None
                            for j in range(EPC):
                                gj = []
                                for m in range(DT):
                                    ps = mps.tile([P, T], F32, name=f"ehp{m}", tag="ehp")
                                    _mm_acc(nc, ps[:], [
                                        (w1[j][:, kt * EH + m * P: kt * EH + (m + 1) * P],
                                         hc[kt][:]) for kt in range(DT)])
                                    gm = mpb.tile([P, T], F32R, name=f"eg{m}", tag="eg",
                                                  bufs=8)
                                    nc.scalar.activation(gm[:], ps[:], AF.Silu,
                                                         bias=B1[:, j * 4 + m:j * 4 + m + 1])
                                    gj.append(gm)
                                cb_ps = mps.tile([P, T], F32, name="cb_ps", tag="gen",
                                                 bufs=1)
                                nc.tensor.matmul(cb_ps[:], ones_r[:],
                                                 cmb[j][0:1, r * T:(r + 1) * T],
                                                 start=True, stop=True)
                                cb = mpb.tile([P, T], F32R, name="cb", tag="cb")
                                nc.scalar.copy(cb[:], cb_ps[:])
                                ynew = []
                                for m in range(DT):
                                    ps = mps.tile([P, T], F32, name=f"eyp{m}", tag="eyp")
                                    _mm_acc(nc, ps[:], [
                                        (w2[j][:, kt * DIM + m * P: kt * DIM + (m + 1) * P],
                                         gj[kt][:]) for kt in range(DT)])
                                    ym = mpb.tile([P, T], F32, name=f"ey{m}", tag="ey",
                                                  bufs=8)
                                    nc.vector.scalar_tensor_tensor(
                                        out=ym[:], in0=ps[:],
                                        scalar=B2[:, j * 4 + m:j * 4 + m + 1],
                                        in1=cb[:], op0=ADD, op1=MUL)
                                    ynew.append(ym)
                                if yacc is None:
                                    yacc = ynew
                                else:
                                    y2 = []
                                    for m in range(DT):
                                        ys = mpb.tile([P, T], F32, name=f"ys{m}",
                                                      tag="ys", bufs=8)
                                        nc.vector.tensor_tensor(ys[:], yacc[m][:],
                                                                ynew[m][:], ADD)
                                        y2.append(ys)
                                    yacc = y2
                            for m in range(DT):
                                nc.sync.dma_start(
                                    rs_in[r:r + 1, m * P * T:(m + 1) * P * T]
                                    .rearrange("a (p f) -> (a p) f", p=P), yacc[m][:])
                        # aux finalize for this layer
                        S_sb = mpb.tile([E, 1], F32R, name="S_sb", tag="S_sb")
                        nc.vector.tensor_copy(S_sb[:], S_ps[:, 0:1])
                        sq16 = mpb.tile([E, 1], F32R, name="sq16", tag="sq16")
                        nc.vector.tensor_tensor(sq16[:], S_sb[:], S_sb[:], MUL)
                        aux_ps = mps.tile([1, 8], F32, name="aux_ps", tag="gen", bufs=1)
                        nc.tensor.matmul(aux_ps[:], sq16[:], ones8r[0:E, :],
                                         start=True, stop=True)
                        aux_new = cpool.tile([1, 1], F32, name=f"aux{l}", tag="auxn",
                                             bufs=2)
                        nc.vector.tensor_tensor(aux_new[:], aux_acc[:],
                                                aux_ps[0:1, 0:1], ADD)
                        aux_acc = aux_new

                        # reduce-scatter of partial outputs
                        rs_out = dloc.tile([1, H_ELEMS], F32, name="rs_out",
                                           tag="rs_out")
                        nc.gpsimd.collective_compute(
                            "ReduceScatter", ADD, replica_groups=RG,
                            ins=[rs_in[:]], outs=[rs_out[:]])
                        xn = []
                        for dt in range(DT):
                            rsb = mpb.tile([P, T], F32, name=f"rsb{dt}", tag="rsb")
                            nc.sync.dma_start(
                                rsb[:],
                                rs_out[0:1, dt * P * T:(dt + 1) * P * T]
                                .rearrange("a (p f) -> (a p) f", p=P))
                            nx = xpool.tile([P, T], F32, name=f"x{l}m{dt}", tag="x")
                            nc.vector.tensor_tensor(nx[:], x[dt][:], rsb[:], ADD)
                            xn.append(nx)
                        x = xn

            # ---------------- final norm + lm_head ----------------
            with (
                tc.tile_pool(name="lm", bufs=1) as lp,
                tc.tile_pool(name="lm_b", bufs=2) as lpb,
                tc.tile_pool(name="lm_ev", bufs=8) as lev,
                tc.tile_pool(name="lm_ps", bufs=4, space="PSUM") as lps,
            ):
                xf = rmsnorm(x, d_normf, lpb, lps, "nf")
                hf_in = dloc.tile([1, H_ELEMS], F32R, name="hf_in", tag="hf_in")
                for dt in range(DT):
                    nc.sync.dma_start(
                        hf_in[0:1, dt * P * T:(dt + 1) * P * T]
                        .rearrange("a (p f) -> (a p) f", p=P), xf[dt][:])
                hf_ag = dsh.tile([NCORES, H_ELEMS], F32R, addr_space="Shared",
                                 name="hf_ag", tag="hf_ag")
                nc.gpsimd.collective_compute(
                    "AllGather", mybir.AluOpType.bypass, replica_groups=RG,
                    ins=[hf_in[:]], outs=[hf_ag[:]])
                hfc = []
                for r in range(NCORES):
                    for kt in range(DT):
                        hx = lp.tile([P, T], F32R, name=f"hf{r}_{kt}")
                        nc.sync.dma_start(
                            hx[:],
                            hf_ag[r:r + 1, kt * P * T:(kt + 1) * P * T]
                            .rearrange("a (p f) -> (a p) f", p=P))
                        hfc.append(hx)
                # vocab chunks
                vchunks = []
                v0 = 0
                while v0 < VS:
                    vw_ = min(512, VS - v0)
                    vchunks.append((v0, vw_))
                    v0 += vw_
                for (v0, vw_) in vchunks:
                    ev = []
                    for kt in range(DT):
                        e = lev.tile([P, 512], F32R, name=f"ev{kt}", tag="ev")
                        nc.sync.dma_start(e[:, 0:vw_],
                                          d_embTv[kt * P:(kt + 1) * P, v0:v0 + vw_])
                        ev.append(e)
                    for r in range(NCORES):
                        for th in range(2):
                            ps = lps.tile([P, 512], F32, name="lmps", tag="lmps")
                            _mm_acc(nc, ps[:, 0:vw_], [
                                (hfc[r * DT + kt][:, th * P:(th + 1) * P],
                                 ev[kt][:, 0:vw_]) for kt in range(DT)])
                            osb = lpb.tile([P, 512], F32, name="osb", tag="osb",
                                           bufs=6)
                            nc.vector.tensor_copy(osb[:, 0:vw_], ps[:, 0:vw_])
                            nc.sync.dma_start(
                                d_logits[r * T + th * P: r * T + (th + 1) * P,
                                         v0:v0 + vw_], osb[:, 0:vw_])
                # aux output
                aux_f = lpb.tile([1, 1], F32, name="aux_f")
                nc.scalar.activation(aux_f[:], aux_acc[:], AF.Copy,
                                     scale=1e-5 / E)
                nc.sync.dma_start(d_aux[:], aux_f[:])

    nc.compile()
    return nc


# ---------------- host side ----------------

def _theta_tables():
    theta = 1.0 / (10000.0 ** (np.arange(0, ROPE_DIM, 2, dtype=np.float32) / ROPE_DIM))
    pos = np.arange(S, dtype=np.float32)
    ang = pos[:, None] * theta[None, :]          # [S,16]
    ang2 = np.concatenate([ang, ang], axis=-1)   # [S,32]
    c = ang2[:, ::2].astype(np.float32)          # [S,16]
    sn = ang2[:, 1::2].astype(np.float32)
    return c, sn


def _bias_cols(b):
    # [K*128] -> [128, K] with col kt = dims kt*128..(kt+1)*128
    k = b.shape[0] // P
    return np.ascontiguousarray(b.reshape(k, P).T.astype(np.float32))


# head permutation: fm tile m holds head m (partitions 0-63, kv head 0) and
# head m+4 (partitions 64-127, kv head 1)
_HEAD_PERM = np.concatenate(
    [np.arange((m + 4 * half) * HD, (m + 4 * half + 1) * HD)
     for m in range(4) for half in range(2)])


def _band_mask(window, c):
    # maskT [S keys, T queries] for core c's queries, additive
    q = (c % 4) * T + np.arange(T)
    k = np.arange(S)
    ok = (k[:, None] <= q[None, :]) & (k[:, None] >= q[None, :] - (window - 1))
    return np.where(ok, 0.0, FP16_MIN).astype(np.float32)


def prepare_in_maps(input_ids, params):
    ids_flat = np.asarray(input_ids).reshape(-1).astype(np.int32)
    pr = params
    emb = np.asarray(pr["emb"], np.float32)
    c_tab, s_tab = _theta_tables()
    in_maps = []
    for c in range(NCORES):
        m = {}
        loc = ids_flat[c * T:(c + 1) * T]
        m["ids"] = np.ascontiguousarray(loc.reshape(2, P).T)  # [128,2]
        m["emb"] = emb
        pos0 = (c % 4) * T
        ct = c_tab[pos0:pos0 + T]  # [256,16]
        st = s_tab[pos0:pos0 + T]
        m["ropec"] = np.ascontiguousarray(
            ct.reshape(2, P, 16).transpose(1, 0, 2).reshape(P, 32))
        m["ropes"] = np.ascontiguousarray(
            st.reshape(2, P, 16).transpose(1, 0, 2).reshape(P, 32))
        m["maskg"] = _band_mask(10000, c)
        m["masks"] = _band_mask(64, c)
        m["embTv"] = np.ascontiguousarray(emb[c * VS:(c + 1) * VS].T)
        m["normf"] = _bias_cols(np.asarray(pr["norm_f"], np.float32))
        for l, bp in enumerate(pr["blocks"]):
            g = lambda k: np.asarray(bp[k], np.float32)
            m[f"L{l}_n1"] = _bias_cols(g("norm1"))
            m[f"L{l}_n2"] = _bias_cols(g("norm2"))
            m[f"L{l}_qw"] = np.ascontiguousarray(g("q_w")[:, _HEAD_PERM])
            m[f"L{l}_qb"] = _bias_cols(g("q_b")[_HEAD_PERM] / 8.0)
            m[f"L{l}_kw"] = g("k_w")
            m[f"L{l}_kb"] = g("k_b").reshape(P, 1)
            m[f"L{l}_vw"] = g("v_w")
            m[f"L{l}_vb"] = g("v_b").reshape(P, 1)
            m[f"L{l}_ow"] = g("o_w")
            m[f"L{l}_ob"] = _bias_cols(g("o_b"))
            m[f"L{l}_sink"] = g("sink").reshape(1, 1)
            if "w1" in bp:
                m[f"L{l}_w1"] = g("w1")
                m[f"L{l}_b1"] = _bias_cols(g("b1"))
                m[f"L{l}_w2"] = g("w2")
                m[f"L{l}_b2"] = _bias_cols(g("b2"))
            else:
                m[f"L{l}_rw"] = g("router_w") / 0.1
                m[f"L{l}_rb"] = (g("router_b") / 0.1).reshape(1, E)
                e0 = c * EPC
                m[f"L{l}_W1"] = np.ascontiguousarray(
                    g("W1")[e0:e0 + EPC].reshape(EPC * DIM, EH))
                m[f"L{l}_B1"] = np.concatenate(
                    [_bias_cols(g("b1e")[e0 + j]) for j in range(EPC)], axis=1)
                m[f"L{l}_W2"] = np.ascontiguousarray(
                    g("W2")[e0:e0 + EPC].reshape(EPC * EH, DIM))
                m[f"L{l}_B2"] = np.concatenate(
                    [_bias_cols(g("b2e")[e0 + j]) for j in range(EPC)], axis=1)
        in_maps.append(m)
    return in_maps


_NC_CACHE = {}


def get_nc():
    if "nc" not in _NC_CACHE:
        _NC_CACHE["nc"] = build_nc()
    return _NC_CACHE["nc"]


def run(input_ids, params, trace=False, **kw):
    nc = get_nc()
    in_maps = prepare_in_maps(input_ids, params)
    res = bass_utils.run_bass_kernel_spmd(
        nc, in_maps, core_ids=list(range(NCORES)), trace=trace, **kw)
    logits = np.concatenate([res.results[c]["logits"] for c in range(NCORES)],
                            axis=1)
    logits = logits.reshape(B, S, V)
    aux = np.float32(res.results[0]["aux"].reshape(()))
    return (logits, aux), res


def kernel(input_ids, params):
    out, _ = run(input_ids, params)
    return out


# revision 61
# speedup vs baseline: 1.0360x; 1.0360x over previous
"""Trainium2 Bass kernel for nn_MiMoV2Flash (7-layer MoE transformer fwd).

Sharding (8 NeuronCores):
  - tokens (B*S = 2048) sequence-parallel: core c owns tokens [c*256, (c+1)*256)
    (batch 0 -> cores 0-3, batch 1 -> cores 4-7)
  - attention: AllGather of per-core K/V each layer; per-core masks encode
    causal/sliding-window structure
  - MoE: expert-parallel (2 experts/core); AllGather of normed hidden,
    dense dispatch weighted by top-2 combine, ReduceScatter of partial outputs
  - lm_head: vocab-sharded (4000 vocab rows/core), AllGather of final hidden
Matmuls run as float32r (full-rate), residual stream kept in fp32.
"""
import sys

if "/opt/trn_rl_repo" not in sys.path:
    sys.path.insert(0, "/opt/trn_rl_repo")

import numpy as np

import concourse.bass as bass
import concourse.bacc as bacc
import concourse.mybir as mybir
from concourse import tile
from concourse import bass_utils
from concourse.masks import make_identity

# ---- model constants (hardcoded per problem spec) ----
NCORES = 8
DIM = 512
NH = 8
NKV = 2
HD = 64
E = 16
EPC = E // NCORES  # experts per core
EH = 512
V = 32000
VS = V // NCORES  # vocab slice per core
B = 2
S = 1024
N = B * S            # 2048 tokens
T = N // NCORES      # 256 tokens per core
P = 128
DT = DIM // P        # 4 feature tiles
ROPE_DIM = 32
FP16_MIN = float(np.finfo(np.float16).min)
EPS = 1e-6
LAYER_SPECS = [("dense", 10000), ("moe", 64), ("moe", 64), ("moe", 10000),
               ("moe", 64), ("moe", 64), ("moe", 10000)]

F32 = mybir.dt.float32
F32R = mybir.dt.float32r
BF16 = mybir.dt.bfloat16
I32 = mybir.dt.int32
ADD = mybir.AluOpType.add
SUB = mybir.AluOpType.subtract
MUL = mybir.AluOpType.mult
ISEQ = mybir.AluOpType.is_equal
AX = mybir.AxisListType.X
AF = mybir.ActivationFunctionType

KV_ELEMS = P * T + T * P          # k_fm [128,256] + v_tm [256,128] = 65536
H_ELEMS = DIM * T                 # 131072


def _mm_acc(nc, out_ap, pairs):
    n = len(pairs)
    for i, (l, r) in enumerate(pairs):
        nc.tensor.matmul(out_ap, l, r, start=(i == 0), stop=(i == n - 1))


def build_nc():
    nc = bacc.Bacc("TRN2", target_bir_lowering=False, debug=False,
                   num_devices=NCORES)
    RG = [list(range(NCORES))]

    # ---------------- DRAM I/O ----------------
    d_ids = nc.dram_tensor("ids", [P, 2], I32, kind="ExternalInput")
    d_emb = nc.dram_tensor("emb", [V, DIM], F32, kind="ExternalInput")
    d_ropec = nc.dram_tensor("ropec", [P, 32], F32, kind="ExternalInput")
    d_ropes = nc.dram_tensor("ropes", [P, 32], F32, kind="ExternalInput")
    d_maskg = nc.dram_tensor("maskg", [S, T], F32, kind="ExternalInput")
    d_masks = nc.dram_tensor("masks", [S, T], F32, kind="ExternalInput")
    d_embTv = nc.dram_tensor("embTv", [DIM, VS], F32R, kind="ExternalInput")
    d_normf = nc.dram_tensor("normf", [P, DT], F32, kind="ExternalInput")
    dL = []
    for l, (kind, _w) in enumerate(LAYER_SPECS):
        t = {}
        t["n1"] = nc.dram_tensor(f"L{l}_n1", [P, DT], F32, kind="ExternalInput")
        t["n2"] = nc.dram_tensor(f"L{l}_n2", [P, DT], F32, kind="ExternalInput")
        t["qw"] = nc.dram_tensor(f"L{l}_qw", [DIM, DIM], F32R, kind="ExternalInput")
        t["qb"] = nc.dram_tensor(f"L{l}_qb", [P, DT], F32, kind="ExternalInput")  # pre-scaled by 1/8
        t["kw"] = nc.dram_tensor(f"L{l}_kw", [DIM, P], F32R, kind="ExternalInput")
        t["kb"] = nc.dram_tensor(f"L{l}_kb", [P, 1], F32, kind="ExternalInput")
        t["vw"] = nc.dram_tensor(f"L{l}_vw", [DIM, P], F32R, kind="ExternalInput")
        t["vb"] = nc.dram_tensor(f"L{l}_vb", [P, 1], F32, kind="ExternalInput")
        t["ow"] = nc.dram_tensor(f"L{l}_ow", [DIM, DIM], F32R, kind="ExternalInput")
        t["ob"] = nc.dram_tensor(f"L{l}_ob", [P, DT], F32, kind="ExternalInput")
        t["sink"] = nc.dram_tensor(f"L{l}_sink", [1, 1], F32, kind="ExternalInput")
        if kind == "dense":
            t["w1"] = nc.dram_tensor(f"L{l}_w1", [DIM, 2048], F32R, kind="ExternalInput")
            t["b1"] = nc.dram_tensor(f"L{l}_b1", [P, 16], F32, kind="ExternalInput")
            t["w2"] = nc.dram_tensor(f"L{l}_w2", [2048, DIM], F32R, kind="ExternalInput")
            t["b2"] = nc.dram_tensor(f"L{l}_b2", [P, DT], F32, kind="ExternalInput")
        else:
            t["rw"] = nc.dram_tensor(f"L{l}_rw", [DIM, E], F32R, kind="ExternalInput")  # /0.1 folded
            t["rb"] = nc.dram_tensor(f"L{l}_rb", [1, E], F32R, kind="ExternalInput")
            t["W1"] = nc.dram_tensor(f"L{l}_W1", [EPC * DIM, EH], F32R, kind="ExternalInput")
            t["B1"] = nc.dram_tensor(f"L{l}_B1", [P, EPC * 4], F32, kind="ExternalInput")
            t["W2"] = nc.dram_tensor(f"L{l}_W2", [EPC * EH, DIM], F32R, kind="ExternalInput")
            t["B2"] = nc.dram_tensor(f"L{l}_B2", [P, EPC * 4], F32, kind="ExternalInput")
        dL.append(t)

    d_logits = nc.dram_tensor("logits", [N, VS], F32, kind="ExternalOutput")
    d_aux = nc.dram_tensor("aux", [1, 1], F32, kind="ExternalOutput")

    with tile.TileContext(nc) as tc:
        pid = nc.partition_id()
        b4 = (pid // 4) * 4  # first core of my batch

        # ---------------- persistent pools ----------------
        with (
            tc.tile_pool(name="const", bufs=1) as cpool,
            tc.tile_pool(name="xres", bufs=12) as xpool,
            tc.tile_pool(name="dram_loc", bufs=2, space="DRAM") as dloc,
            tc.tile_pool(name="dram_sh", bufs=2, space="DRAM") as dsh,
        ):
            ones32c = cpool.tile([P, 1], F32, name="ones32c")
            nc.gpsimd.memset(ones32c[:], 1.0)
            ones32r = cpool.tile([1, P], F32, name="ones32r")
            nc.gpsimd.memset(ones32r[:], 1.0)
            ones_c = cpool.tile([P, 1], F32R, name="ones_c")
            nc.vector.tensor_copy(ones_c[:], ones32c[:])
            ones_r = cpool.tile([1, P], F32R, name="ones_r")
            nc.vector.tensor_copy(ones_r[:], ones32r[:])
            ones32w = cpool.tile([P, 8], F32, name="ones32w")
            nc.gpsimd.memset(ones32w[:], 1.0)
            ones8r = cpool.tile([P, 8], F32R, name="ones8r")
            nc.vector.tensor_copy(ones8r[:], ones32w[:])
            id32 = cpool.tile([P, P], F32, name="id32")
            make_identity(nc, id32[:])
            id32r = cpool.tile([P, P], F32R, name="id32r")
            nc.vector.tensor_copy(id32r[:], id32[:])
            maskg_sb = cpool.tile([P, 8 * T], F32, name="maskg_sb")
            masks_sb = cpool.tile([P, 8 * T], F32, name="masks_sb")
            for kt in range(8):
                nc.sync.dma_start(maskg_sb[:, kt * T:(kt + 1) * T],
                                  d_maskg[kt * P:(kt + 1) * P, :])
                nc.sync.dma_start(masks_sb[:, kt * T:(kt + 1) * T],
                                  d_masks[kt * P:(kt + 1) * P, :])
            aux_acc = cpool.tile([1, 1], F32, name="aux_acc")
            nc.vector.memset(aux_acc[:], 0.0)
            eps_c = cpool.tile([1, 1], F32, name="eps_c")
            nc.scalar.mul(eps_c[:], ones32c[0:1, :], EPS)

            # ---------------- embedding + rope ----------------
            x = [None] * DT  # feature-major residual [128, 256] fp32, row=dt*128+p
            with (
                tc.tile_pool(name="emb_sb", bufs=2) as epool,
                tc.tile_pool(name="emb_ps", bufs=4, space="PSUM") as epps,
            ):
                ids_sb = epool.tile([P, 2], I32, name="ids_sb")
                nc.sync.dma_start(ids_sb[:], d_ids[:])
                g_t = []
                for t in range(2):
                    g = epool.tile([P, DIM], F32, name=f"g{t}", tag="g")
                    nc.gpsimd.indirect_dma_start(
                        out=g[:], out_offset=None, in_=d_emb[:],
                        in_offset=bass.IndirectOffsetOnAxis(ap=ids_sb[:, t:t + 1], axis=0),
                    )
                    g_t.append(g)
                ropec_sb = epool.tile([P, 32], F32, name="ropec_sb")
                ropes_sb = epool.tile([P, 32], F32, name="ropes_sb")
                nc.sync.dma_start(ropec_sb[:], d_ropec[:])
                nc.sync.dma_start(ropes_sb[:], d_ropes[:])
                for t in range(2):
                    g = g_t[t]
                    ct = ropec_sb[:, t * 16:(t + 1) * 16]
                    st = ropes_sb[:, t * 16:(t + 1) * 16]
                    xe = g[:, 0:ROPE_DIM:2]
                    xo = g[:, 1:ROPE_DIM:2]
                    ta = epool.tile([P, 16], F32, name="ta", tag="ta")
                    tb = epool.tile([P, 16], F32, name="tb", tag="tb")
                    ne = epool.tile([P, 16], F32, name="ne", tag="ne")
                    no = epool.tile([P, 16], F32, name="no", tag="no")
                    nc.vector.tensor_tensor(ta[:], xe, ct, MUL)
                    nc.vector.tensor_tensor(tb[:], xo, st, MUL)
                    nc.vector.tensor_tensor(ne[:], ta[:], tb[:], SUB)
                    nc.vector.tensor_tensor(ta[:], xo, ct, MUL)
                    nc.vector.tensor_tensor(tb[:], xe, st, MUL)
                    nc.vector.tensor_tensor(no[:], ta[:], tb[:], ADD)
                    nc.vector.tensor_copy(xe, ne[:])
                    nc.vector.tensor_copy(xo, no[:])
                # transpose token-major [128tok, 512] -> feature-major x[dt][:, t*128]
                for dt in range(DT):
                    x[dt] = xpool.tile([P, T], F32, name=f"x{dt}", tag="x")
                for t in range(2):
                    for dt in range(DT):
                        tp = epps.tile([P, P], F32, name="tp", tag="tp")
                        nc.tensor.transpose(tp[:], g_t[t][:, dt * P:(dt + 1) * P], id32[:])
                        nc.vector.tensor_copy(x[dt][:, t * P:(t + 1) * P], tp[:])

            # ---------------- helpers ----------------
            def rmsnorm(xt, w_dram, pool, psp, name, pstag="gen"):
                """fm rmsnorm: returns 4 f32r tiles [128,256]."""
                w_sb = pool.tile([P, DT], F32, name=f"{name}_w", tag=f"{name}_w")
                nc.sync.dma_start(w_sb[:], w_dram[:])
                sq = []
                for dt in range(DT):
                    s = pool.tile([P, T], F32R, name=f"{name}_sq{dt}",
                                  tag=f"{name}_sq", bufs=4)
                    nc.vector.tensor_tensor(s[:], xt[dt][:], xt[dt][:], MUL)
                    sq.append(s)
                ss = psp.tile([1, T], F32, name=f"{name}_ss", tag=pstag, bufs=1)
                _mm_acc(nc, ss[:], [(ones_c[:], s[:]) for s in sq])
                srow = pool.tile([1, T], F32, name=f"{name}_sr", tag=f"{name}_sr")
                nc.scalar.activation(srow[:], ss[:], AF.Sqrt, bias=eps_c[0:1, 0:1],
                                     scale=1.0 / DIM)
                rrow = pool.tile([1, T], F32R, name=f"{name}_rr", tag=f"{name}_rr")
                with nc.allow_low_precision(reason="f32r feeds full-rate matmul"):
                    nc.vector.reciprocal(rrow[:], srow[:])
                sc = psp.tile([P, T], F32, name=f"{name}_sc", tag=pstag, bufs=1)
                nc.tensor.matmul(sc[:], ones_r[:], rrow[:], start=True, stop=True)
                out = []
                for dt in range(DT):
                    o = pool.tile([P, T], F32R, name=f"{name}_o{dt}",
                                  tag=f"{name}_o", bufs=4)
                    nc.vector.scalar_tensor_tensor(
                        out=o[:], in0=xt[dt][:], scalar=w_sb[:, dt:dt + 1], in1=sc[:],
                        op0=MUL, op1=MUL)
                    out.append(o)
                return out

            def load_w(pool, dram, rows, cols, name, tag=None):
                """Load [rows, cols] dram weight into sbuf [128, (rows//128)*cols],
                col-block kt holds dram rows kt*128..+128."""
                kt_n = rows // P
                w = pool.tile([P, kt_n * cols], F32R, name=name, tag=tag or name)
                for kt in range(kt_n):
                    nc.sync.dma_start(w[:, kt * cols:(kt + 1) * cols],
                                      dram[kt * P:(kt + 1) * P, :])
                return w

            # ---------------- transformer layers ----------------
            for l, (kind, window) in enumerate(LAYER_SPECS):
                t = dL[l]
                mask_sb = maskg_sb if window > S else masks_sb

                # ===== attention =====
                with (
                    tc.tile_pool(name=f"at{l}", bufs=2) as ap,
                    tc.tile_pool(name=f"at{l}_e", bufs=3) as ep,
                    tc.tile_pool(name=f"at{l}_ps", bufs=2, space="PSUM") as aps,
                    tc.tile_pool(name=f"at{l}_ops", bufs=2, space="PSUM") as ops_,
                ):
                    xh = rmsnorm(x, t["n1"], ap, aps, f"n1_{l}")
                    qw = load_w(ap, t["qw"], DIM, DIM, f"qw{l}", tag="qw")
                    kw = load_w(ap, t["kw"], DIM, P, f"kw{l}", tag="kw")
                    vw = load_w(ap, t["vw"], DIM, P, f"vw{l}", tag="vw")
                    qb = ap.tile([P, DT], F32, name=f"qb{l}", tag="qb")
                    kb = ap.tile([P, 1], F32, name=f"kb{l}", tag="kb")
                    vb = ap.tile([P, 1], F32, name=f"vb{l}", tag="vb")
                    sink = ap.tile([1, 1], F32, name=f"sink{l}", tag="sink")
                    nc.sync.dma_start(qb[:], t["qb"][:])
                    nc.sync.dma_start(kb[:], t["kb"][:])
                    nc.sync.dma_start(vb[:], t["vb"][:])
                    nc.sync.dma_start(sink[:], t["sink"][:])

                    # q projection (scaled by 1/8; qb pre-scaled on host)
                    q_sb = []
                    for m in range(DT):
                        ps = aps.tile([P, T], F32, name=f"qp{m}", tag="qkv_ps")
                        _mm_acc(nc, ps[:], [(qw[:, kt * DIM + m * P: kt * DIM + (m + 1) * P],
                                             xh[kt][:]) for kt in range(DT)])
                        q = ap.tile([P, T], F32R, name=f"q{m}", tag="q", bufs=4)
                        nc.scalar.activation(q[:], ps[:], AF.Identity,
                                             bias=qb[:, m:m + 1], scale=0.125)
                        q_sb.append(q)
                    # k, v projections
                    kps = aps.tile([P, T], F32, name="kps", tag="qkv_ps")
                    _mm_acc(nc, kps[:], [(kw[:, kt * P:(kt + 1) * P], xh[kt][:])
                                         for kt in range(DT)])
                    k_sb = ap.tile([P, T], F32R, name="k_sb", tag="k_sb")
                    nc.scalar.activation(k_sb[:], kps[:], AF.Identity, bias=kb[:, 0:1])
                    vps = aps.tile([P, T], F32, name="vps", tag="qkv_ps")
                    _mm_acc(nc, vps[:], [(vw[:, kt * P:(kt + 1) * P], xh[kt][:])
                                         for kt in range(DT)])
                    v_sb = ap.tile([P, T], F32R, name="v_sb", tag="v_sb")
                    nc.scalar.activation(v_sb[:], vps[:], AF.Identity, bias=vb[:, 0:1])
                    # transpose v -> token-major [256, 128] (2 ptiles)
                    vt_sb = ap.tile([P, T], F32R, name="vt_sb", tag="vt_sb")
                    for th in range(2):
                        tp = aps.tile([P, P], F32R, name="vtp", tag="gen", bufs=1)
                        nc.tensor.transpose(tp[:], v_sb[:, th * P:(th + 1) * P], id32r[:])
                        nc.vector.tensor_copy(vt_sb[:, th * P:(th + 1) * P], tp[:])

                    # kv allgather
                    kv_in = dloc.tile([1, KV_ELEMS], F32R, name="kv_in", tag="kv_in")
                    nc.sync.dma_start(
                        kv_in[0:1, 0:P * T].rearrange("a (p f) -> (a p) f", p=P), k_sb[:])
                    for th in range(2):
                        nc.sync.dma_start(
                            kv_in[0:1, P * T + th * P * P: P * T + (th + 1) * P * P]
                            .rearrange("a (p f) -> (a p) f", p=P),
                            vt_sb[:, th * P:(th + 1) * P])
                    kv_ag = dsh.tile([NCORES, KV_ELEMS], F32R, addr_space="Shared",
                                     name="kv_ag", tag="kv_ag")
                    nc.gpsimd.collective_compute(
                        "AllGather", mybir.AluOpType.bypass, replica_groups=RG,
                        ins=[kv_in[:]], outs=[kv_ag[:]])

                    # load this batch's K (fm) and V (tm)
                    ksb = ap.tile([P, 4 * T], F32R, name="ksb", tag="ksb")
                    vsb = ap.tile([P, 8 * P], F32R, name="vsb", tag="vsb")
                    for j in range(4):
                        src = kv_ag[bass.ds(b4 + j, 1), 0:P * T]
                        nc.sync.dma_start(
                            ksb[:, j * T:(j + 1) * T],
                            src.rearrange("a (p f) -> (a p) f", p=P))
                        for th in range(2):
                            srcv = kv_ag[bass.ds(b4 + j, 1),
                                         P * T + th * P * P: P * T + (th + 1) * P * P]
                            nc.sync.dma_start(
                                vsb[:, (j * 2 + th) * P:(j * 2 + th + 1) * P],
                                srcv.rearrange("a (p f) -> (a p) f", p=P))

                    # scores^T / softmax / AV  (key-major).
                    # Head layout is permuted (host-side): q tile m holds head m
                    # on partitions 0-63 and head m+4 on partitions 64-127, so
                    # each head's q shares a base partition with its kv head.
                    # Per-head AV accumulators sit at psum base 0 (v3 requires
                    # matmul dst partition offset 0).
                    esink = ap.tile([1, 1], F32, name="esink", tag="esink")
                    nc.scalar.activation(esink[:], sink[:], AF.Exp)
                    o_heads = []
                    for h in range(NH):
                        mt = h % 4
                        pb = (h // 4) * HD
                        q_ap = q_sb[mt][pb:pb + HD, :]
                        o_ps = ops_.tile([HD, T], F32, name=f"oph{h}", tag="o_ps")
                        den = aps.tile([1, T], F32, name=f"den{h}", tag="den", bufs=1)
                        for kt in range(8):
                            sc = aps.tile([P, T], F32, name=f"sc{kt}", tag="sc")
                            nc.tensor.matmul(
                                sc[:],
                                ksb[pb:pb + HD,
                                    (kt // 2) * T + (kt % 2) * P:
                                    (kt // 2) * T + (kt % 2 + 1) * P],
                                q_ap, start=True, stop=True)
                            ei = ep.tile([P, T], F32, name="ei", tag="ei")
                            nc.vector.tensor_tensor(
                                ei[:], sc[:], mask_sb[:, kt * T:(kt + 1) * T], ADD)
                            e_sb = ep.tile([P, T], F32R, name="e_sb", tag="e_sb")
                            nc.scalar.activation(e_sb[:], ei[:], AF.Exp)
                            nc.tensor.matmul(den[:], ones_c[:], e_sb[:],
                                             start=(kt == 0), stop=(kt == 7))
                            nc.tensor.matmul(
                                o_ps[:],
                                vsb[:, kt * P + pb: kt * P + pb + HD],
                                e_sb[:], start=(kt == 0), stop=(kt == 7))
                        denf = ap.tile([1, T], F32, name=f"denf{h}", tag="denf")
                        nc.vector.tensor_scalar(
                            out=denf[:], in0=den[:],
                            scalar1=esink[0:1, 0:1], scalar2=None, op0=ADD)
                        rec = ap.tile([1, T], F32R, name=f"rec{h}", tag="rec")
                        with nc.allow_low_precision(reason="f32r feeds matmul"):
                            nc.vector.reciprocal(rec[:], denf[:])
                        scb = aps.tile([HD, T], F32, name=f"scb{h}", tag="gen", bufs=1)
                        nc.tensor.matmul(scb[:], ones_r[0:1, 0:HD], rec[:],
                                         start=True, stop=True)
                        bc = ap.tile([HD, T], F32R, name=f"bc{h}", tag="bc")
                        nc.scalar.copy(bc[:], scb[:])
                        o_h = ap.tile([HD, T], F32R, name=f"oh{h}", tag="o_sb",
                                      bufs=8)
                        nc.vector.tensor_tensor(o_h[:], o_ps[:], bc[:], MUL)
                        o_heads.append(o_h)
                    # o-projection: K=64 per head, ow loaded head-major [64, 8*512]
                    ow = ap.tile([HD, NH * DIM], F32R, name=f"ow{l}", tag="ow")
                    for h in range(NH):
                        nc.sync.dma_start(ow[:, h * DIM:(h + 1) * DIM],
                                          t["ow"][h * HD:(h + 1) * HD, :])
                    ob = ap.tile([P, DT], F32, name=f"ob{l}", tag="ob")
                    nc.sync.dma_start(ob[:], t["ob"][:])
                    xn = []
                    for dt in range(DT):
                        ps = aps.tile([P, T], F32, name=f"aop{dt}", tag="qkv_ps")
                        _mm_acc(nc, ps[:], [
                            (ow[:, h * DIM + dt * P: h * DIM + (dt + 1) * P],
                             o_heads[h][:]) for h in range(NH)])
                        nx = xpool.tile([P, T], F32, name=f"x{l}a{dt}", tag="x")
                        nc.vector.scalar_tensor_tensor(
                            out=nx[:], in0=ps[:], scalar=ob[:, dt:dt + 1], in1=x[dt][:],
                            op0=ADD, op1=ADD)
                        xn.append(nx)
                    x = xn

                # ===== FFN =====
                if kind == "dense":
                    with (
                        tc.tile_pool(name=f"ff{l}", bufs=1) as fp,
                        tc.tile_pool(name=f"ff{l}_b", bufs=2) as fpb,
                        tc.tile_pool(name=f"ff{l}_ps", bufs=2, space="PSUM") as fps,
                    ):
                        hh = rmsnorm(x, t["n2"], fpb, fps, f"n2_{l}")
                        w1 = load_w(fp, t["w1"], DIM, 2048, "w1d")
                        w2 = load_w(fp, t["w2"], 2048, DIM, "w2d")
                        b1 = fpb.tile([P, 16], F32, name="b1d")
                        b2 = fpb.tile([P, DT], F32, name="b2d")
                        nc.sync.dma_start(b1[:], t["b1"][:])
                        nc.sync.dma_start(b2[:], t["b2"][:])
                        g = []
                        for m in range(16):
                            ps = fps.tile([P, T], F32, name=f"hp{m}", tag="hp", bufs=3)
                            _mm_acc(nc, ps[:], [
                                (w1[:, kt * 2048 + m * P: kt * 2048 + (m + 1) * P],
                                 hh[kt][:]) for kt in range(DT)])
                            gm = fpb.tile([P, T], F32R, name=f"g{m}", tag="g",
                                          bufs=16)
                            nc.scalar.activation(gm[:], ps[:], AF.Silu,
                                                 bias=b1[:, m:m + 1])
                            g.append(gm)
                        xn = []
                        for dt in range(DT):
                            ps = fps.tile([P, T], F32, name=f"yp{dt}", tag="yp")
                            _mm_acc(nc, ps[:], [
                                (w2[:, m * DIM + dt * P: m * DIM + (dt + 1) * P],
                                 g[m][:]) for m in range(16)])
                            nx = xpool.tile([P, T], F32, name=f"x{l}f{dt}", tag="x")
                            nc.vector.scalar_tensor_tensor(
                                out=nx[:], in0=ps[:], scalar=b2[:, dt:dt + 1],
                                in1=x[dt][:], op0=ADD, op1=ADD)
                            xn.append(nx)
                        x = xn
                else:
                    # ===== MoE =====
                    with (
                        tc.tile_pool(name=f"mo{l}", bufs=1) as mp,
                        tc.tile_pool(name=f"mo{l}_b", bufs=2) as mpb,
                        tc.tile_pool(name=f"mo{l}_h", bufs=8) as mph,
                        tc.tile_pool(name=f"mo{l}_ps", bufs=2, space="PSUM") as mps,
                    ):
                        hh = rmsnorm(x, t["n2"], mpb, mps, f"n2_{l}")
                        h_in = dloc.tile([1, H_ELEMS], F32R, name="h_in", tag="h_in")
                        for dt in range(DT):
                            nc.sync.dma_start(
                                h_in[0:1, dt * P * T:(dt + 1) * P * T]
                                .rearrange("a (p f) -> (a p) f", p=P), hh[dt][:])
                        h_ag = dsh.tile([NCORES, H_ELEMS], F32R, addr_space="Shared",
                                        name="h_ag", tag="h_ag")
                        nc.gpsimd.collective_compute(
                            "AllGather", mybir.AluOpType.bypass, replica_groups=RG,
                            ins=[h_in[:]], outs=[h_ag[:]])

                        w1 = [load_w(mp, t["W1"][j * DIM:(j + 1) * DIM, :], DIM, EH,
                                     f"w1e{j}") for j in range(EPC)]
                        w2 = [load_w(mp, t["W2"][j * EH:(j + 1) * EH, :], EH, DIM,
                                     f"w2e{j}") for j in range(EPC)]
                        rw = mpb.tile([P, DT * E], F32R, name="rw", tag="rw")
                        for kt in range(DT):
                            nc.scalar.dma_start(rw[:, kt * E:(kt + 1) * E],
                                                t["rw"][kt * P:(kt + 1) * P, :])
                        rb = mpb.tile([1, E], F32R, name="rb", tag="rb")
                        B1 = mpb.tile([P, EPC * 4], F32, name="B1", tag="B1")
                        B2 = mpb.tile([P, EPC * 4], F32, name="B2", tag="B2")
                        nc.sync.dma_start(rb[:], t["rb"][:])
                        nc.sync.dma_start(B1[:], t["B1"][:])
                        nc.sync.dma_start(B2[:], t["B2"][:])

                        cmb = [mpb.tile([1, N], F32R, name=f"cmb{j}", tag=f"cmb{j}",
                                        bufs=1) for j in range(EPC)]
                        S_ps = mps.tile([E, 8], F32, name="S_ps", tag="S_ps", bufs=1)
                        rs_in = dloc.tile([NCORES, H_ELEMS], F32, name="rs_in",
                                          tag="rs_in")
                        T2 = 2 * T
                        for r in range(0, NCORES, 2):
                            hc = []
                            for kt in range(DT):
                                hcx = mph.tile([P, T2], F32R, name=f"hc{kt}", tag="hc")
                                for rr in range(2):
                                    nc.scalar.dma_start(
                                        hcx[:, rr * T:(rr + 1) * T],
                                        h_ag[r + rr:r + rr + 1,
                                             kt * P * T:(kt + 1) * P * T]
                                        .rearrange("a (p f) -> (a p) f", p=P))
                                hc.append(hcx)
                            # router for 4 token-tiles
                            for th in range(4):
                                lg_ps = mps.tile([P, E], F32, name="lg_ps", tag="lg_ps")
                                for kt in range(DT):
                                    nc.tensor.matmul(
                                        lg_ps[:], hc[kt][:, th * P:(th + 1) * P],
                                        rw[:, kt * E:(kt + 1) * E],
                                        start=(kt == 0), stop=False)
                                nc.tensor.matmul(lg_ps[:], ones_r[:], rb[:],
                                                 start=False, stop=True)
                                lg = mpb.tile([P, E], F32, name="lg", tag="lg")
                                nc.vector.tensor_copy(lg[:], lg_ps[:])
                                m8 = mpb.tile([P, 8], F32, name="m8", tag="m8")
                                nc.vector.max(m8[:], lg[:])
                                dd = mpb.tile([P, 1], F32, name="dd", tag="dd")
                                nc.vector.tensor_tensor(dd[:], m8[:, 1:2], m8[:, 0:1], SUB)
                                w2s = mpb.tile([P, 1], F32, name="w2s", tag="w2s")
                                nc.scalar.activation(w2s[:], dd[:], AF.Sigmoid)
                                w1s = mpb.tile([P, 1], F32, name="w1s", tag="w1s")
                                nc.vector.tensor_scalar(out=w1s[:], in0=w2s[:],
                                                        scalar1=-1.0, scalar2=1.0,
                                                        op0=MUL, op1=ADD)
                                cmb_tm = mpb.tile([P, EPC], F32, name="cmb_tm",
                                                  tag="cmb_tm")
                                for j in range(EPC):
                                    ecol = pid * EPC + j
                                    lcol = lg[:, bass.ds(ecol, 1)]
                                    eq1 = mpb.tile([P, 1], F32, name="eq1", tag="eq1")
                                    eq2 = mpb.tile([P, 1], F32, name="eq2", tag="eq2")
                                    nc.vector.tensor_tensor(eq1[:], lcol, m8[:, 0:1], ISEQ)
                                    nc.vector.tensor_tensor(eq2[:], lcol, m8[:, 1:2], ISEQ)
                                    t1 = mpb.tile([P, 1], F32, name="t1", tag="t1")
                                    nc.vector.tensor_scalar(
                                        out=t1[:], in0=eq1[:], scalar1=w1s[:, 0:1],
                                        scalar2=None, op0=MUL)
                                    # cmb[:, j] = eq2*w2 + eq1*w1
                                    nc.vector.scalar_tensor_tensor(
                                        out=cmb_tm[:, j:j + 1], in0=eq2[:],
                                        scalar=w2s[:, 0:1], in1=t1[:], op0=MUL, op1=ADD)
                                # transpose cmb_tm columns -> [1, 128] rows
                                for j in range(EPC):
                                    ct = mps.tile([1, P], F32, name="ct", tag="gen",
                                                  bufs=1)
                                    nc.tensor.transpose(ct[:], cmb_tm[:, j:j + 1],
                                                        id32[:])
                                    nc.vector.tensor_copy(
                                        cmb[j][0:1, r * T + th * P:
                                               r * T + (th + 1) * P], ct[:])
                                # aux: probs
                                e16 = mpb.tile([P, E], F32R, name="e16", tag="e16")
                                nc.scalar.activation(e16[:], lg[:], AF.Exp)
                                srow = mpb.tile([P, 1], F32, name="esum", tag="esum")
                                nc.vector.tensor_reduce(srow[:], e16[:], AX, ADD)
                                rr = mpb.tile([P, 1], F32, name="rr", tag="rr")
                                nc.vector.reciprocal(rr[:], srow[:])
                                probs = mpb.tile([P, E], F32R, name="probs", tag="probs")
                                nc.vector.tensor_scalar(out=probs[:], in0=e16[:],
                                                        scalar1=rr[:, 0:1], scalar2=None,
                                                        op0=MUL)
                                nc.tensor.matmul(S_ps[:], probs[:], ones8r[:],
                                                 start=(r == 0 and th == 0),
                                                 stop=(r == NCORES - 2 and th == 3))
                            # experts (dense dispatch, 512-token chunks)
                            yacc = None
                            for j in range(EPC):
                                gj = []
                                for m in range(DT):
                                    ps = mps.tile([P, T2], F32, name=f"ehp{m}", tag="ehp")
                                    _mm_acc(nc, ps[:], [
                                        (w1[j][:, kt * EH + m * P: kt * EH + (m + 1) * P],
                                         hc[kt][:]) for kt in range(DT)])
                                    gm = mpb.tile([P, T2], F32R, name=f"eg{m}", tag="eg",
                                                  bufs=8)
                                    nc.scalar.activation(gm[:], ps[:], AF.Silu,
                                                         bias=B1[:, j * 4 + m:j * 4 + m + 1])
                                    gj.append(gm)
                                cb_ps = mps.tile([P, T2], F32, name="cb_ps", tag="gen",
                                                 bufs=1)
                                nc.tensor.matmul(cb_ps[:], ones_r[:],
                                                 cmb[j][0:1, r * T:(r + 2) * T],
                                                 start=True, stop=True)
                                cb = mpb.tile([P, T2], F32R, name="cb", tag="cb")
                                nc.scalar.copy(cb[:], cb_ps[:])
                                ynew = []
                                for m in range(DT):
                                    ps = mps.tile([P, T2], F32, name=f"eyp{m}", tag="eyp")
                                    _mm_acc(nc, ps[:], [
                                        (w2[j][:, kt * DIM + m * P: kt * DIM + (m + 1) * P],
                                         gj[kt][:]) for kt in range(DT)])
                                    ym = mpb.tile([P, T2], F32, name=f"ey{m}", tag="ey",
                                                  bufs=8)
                                    nc.vector.scalar_tensor_tensor(
                                        out=ym[:], in0=ps[:],
                                        scalar=B2[:, j * 4 + m:j * 4 + m + 1],
                                        in1=cb[:], op0=ADD, op1=MUL)
                                    ynew.append(ym)
                                if yacc is None:
                                    yacc = ynew
                                else:
                                    y2 = []
                                    for m in range(DT):
                                        ys = mpb.tile([P, T2], F32, name=f"ys{m}",
                                                      tag="ys", bufs=8)
                                        nc.vector.tensor_tensor(ys[:], yacc[m][:],
                                                                ynew[m][:], ADD)
                                        y2.append(ys)
                                    yacc = y2
                            for m in range(DT):
                                for rr in range(2):
                                    nc.sync.dma_start(
                                        rs_in[r + rr:r + rr + 1,
                                              m * P * T:(m + 1) * P * T]
                                        .rearrange("a (p f) -> (a p) f", p=P),
                                        yacc[m][:, rr * T:(rr + 1) * T])
                        # aux finalize for this layer
                        S_sb = mpb.tile([E, 1], F32R, name="S_sb", tag="S_sb")
                        nc.vector.tensor_copy(S_sb[:], S_ps[:, 0:1])
                        sq16 = mpb.tile([E, 1], F32R, name="sq16", tag="sq16")
                        nc.vector.tensor_tensor(sq16[:], S_sb[:], S_sb[:], MUL)
                        aux_ps = mps.tile([1, 8], F32, name="aux_ps", tag="gen", bufs=1)
                        nc.tensor.matmul(aux_ps[:], sq16[:], ones8r[0:E, :],
                                         start=True, stop=True)
                        aux_new = cpool.tile([1, 1], F32, name=f"aux{l}", tag="auxn",
                                             bufs=2)
                        nc.vector.tensor_tensor(aux_new[:], aux_acc[:],
                                                aux_ps[0:1, 0:1], ADD)
                        aux_acc = aux_new

                        # reduce-scatter of partial outputs
                        rs_out = dloc.tile([1, H_ELEMS], F32, name="rs_out",
                                           tag="rs_out")
                        nc.gpsimd.collective_compute(
                            "ReduceScatter", ADD, replica_groups=RG,
                            ins=[rs_in[:]], outs=[rs_out[:]])
                        xn = []
                        for dt in range(DT):
                            rsb = mpb.tile([P, T], F32, name=f"rsb{dt}", tag="rsb")
                            nc.sync.dma_start(
                                rsb[:],
                                rs_out[0:1, dt * P * T:(dt + 1) * P * T]
                                .rearrange("a (p f) -> (a p) f", p=P))
                            nx = xpool.tile([P, T], F32, name=f"x{l}m{dt}", tag="x")
                            nc.vector.tensor_tensor(nx[:], x[dt][:], rsb[:], ADD)
                            xn.append(nx)
                        x = xn

            # ---------------- final norm + lm_head ----------------
            with (
                tc.tile_pool(name="lm", bufs=1) as lp,
                tc.tile_pool(name="lm_b", bufs=2) as lpb,
                tc.tile_pool(name="lm_ev", bufs=8) as lev,
                tc.tile_pool(name="lm_ps", bufs=4, space="PSUM") as lps,
            ):
                xf = rmsnorm(x, d_normf, lpb, lps, "nf")
                hf_in = dloc.tile([1, H_ELEMS], F32R, name="hf_in", tag="hf_in")
                for dt in range(DT):
                    nc.sync.dma_start(
                        hf_in[0:1, dt * P * T:(dt + 1) * P * T]
                        .rearrange("a (p f) -> (a p) f", p=P), xf[dt][:])
                hf_ag = dsh.tile([NCORES, H_ELEMS], F32R, addr_space="Shared",
                                 name="hf_ag", tag="hf_ag")
                nc.gpsimd.collective_compute(
                    "AllGather", mybir.AluOpType.bypass, replica_groups=RG,
                    ins=[hf_in[:]], outs=[hf_ag[:]])
                hfc = []
                for r in range(NCORES):
                    for kt in range(DT):
                        hx = lp.tile([P, T], F32R, name=f"hf{r}_{kt}")
                        nc.scalar.dma_start(
                            hx[:],
                            hf_ag[r:r + 1, kt * P * T:(kt + 1) * P * T]
                            .rearrange("a (p f) -> (a p) f", p=P))
                        hfc.append(hx)
                # vocab chunks
                vchunks = []
                v0 = 0
                while v0 < VS:
                    vw_ = min(512, VS - v0)
                    vchunks.append((v0, vw_))
                    v0 += vw_
                for (v0, vw_) in vchunks:
                    ev = []
                    for kt in range(DT):
                        e = lev.tile([P, 512], F32R, name=f"ev{kt}", tag="ev")
                        nc.scalar.dma_start(e[:, 0:vw_],
                                            d_embTv[kt * P:(kt + 1) * P, v0:v0 + vw_])
                        ev.append(e)
                    for r in range(NCORES):
                        for th in range(2):
                            ps = lps.tile([P, 512], F32, name="lmps", tag="lmps")
                            _mm_acc(nc, ps[:, 0:vw_], [
                                (hfc[r * DT + kt][:, th * P:(th + 1) * P],
                                 ev[kt][:, 0:vw_]) for kt in range(DT)])
                            osb = lpb.tile([P, 512], F32, name="osb", tag="osb",
                                           bufs=6)
                            nc.vector.tensor_copy(osb[:, 0:vw_], ps[:, 0:vw_])
                            nc.scalar.dma_start(
                                d_logits[r * T + th * P: r * T + (th + 1) * P,
                                         v0:v0 + vw_], osb[:, 0:vw_])
                # aux output
                aux_f = lpb.tile([1, 1], F32, name="aux_f")
                nc.scalar.activation(aux_f[:], aux_acc[:], AF.Copy,
                                     scale=1e-5 / E)
                nc.sync.dma_start(d_aux[:], aux_f[:])

    nc.compile()
    return nc


# ---------------- host side ----------------

def _theta_tables():
    theta = 1.0 / (10000.0 ** (np.arange(0, ROPE_DIM, 2, dtype=np.float32) / ROPE_DIM))
    pos = np.arange(S, dtype=np.float32)
    ang = pos[:, None] * theta[None, :]          # [S,16]
    ang2 = np.concatenate([ang, ang], axis=-1)   # [S,32]
    c = ang2[:, ::2].astype(np.float32)          # [S,16]
    sn = ang2[:, 1::2].astype(np.float32)
    return c, sn


def _bias_cols(b):
    # [K*128] -> [128, K] with col kt = dims kt*128..(kt+1)*128
    k = b.shape[0] // P
    return np.ascontiguousarray(b.reshape(k, P).T.astype(np.float32))


# head permutation: fm tile m holds head m (partitions 0-63, kv head 0) and
# head m+4 (partitions 64-127, kv head 1)
_HEAD_PERM = np.concatenate(
    [np.arange((m + 4 * half) * HD, (m + 4 * half + 1) * HD)
     for m in range(4) for half in range(2)])


def _band_mask(window, c):
    # maskT [S keys, T queries] for core c's queries, additive
    q = (c % 4) * T + np.arange(T)
    k = np.arange(S)
    ok = (k[:, None] <= q[None, :]) & (k[:, None] >= q[None, :] - (window - 1))
    return np.where(ok, 0.0, FP16_MIN).astype(np.float32)


def prepare_in_maps(input_ids, params):
    ids_flat = np.asarray(input_ids).reshape(-1).astype(np.int32)
    pr = params
    emb = np.asarray(pr["emb"], np.float32)
    c_tab, s_tab = _theta_tables()
    in_maps = []
    for c in range(NCORES):
        m = {}
        loc = ids_flat[c * T:(c + 1) * T]
        m["ids"] = np.ascontiguousarray(loc.reshape(2, P).T)  # [128,2]
        m["emb"] = emb
        pos0 = (c % 4) * T
        ct = c_tab[pos0:pos0 + T]  # [256,16]
        st = s_tab[pos0:pos0 + T]
        m["ropec"] = np.ascontiguousarray(
            ct.reshape(2, P, 16).transpose(1, 0, 2).reshape(P, 32))
        m["ropes"] = np.ascontiguousarray(
            st.reshape(2, P, 16).transpose(1, 0, 2).reshape(P, 32))
        m["maskg"] = _band_mask(10000, c)
        m["masks"] = _band_mask(64, c)
        m["embTv"] = np.ascontiguousarray(emb[c * VS:(c + 1) * VS].T)
        m["normf"] = _bias_cols(np.asarray(pr["norm_f"], np.float32))
        for l, bp in enumerate(pr["blocks"]):
            g = lambda k: np.asarray(bp[k], np.float32)
            m[f"L{l}_n1"] = _bias_cols(g("norm1"))
            m[f"L{l}_n2"] = _bias_cols(g("norm2"))
            m[f"L{l}_qw"] = np.ascontiguousarray(g("q_w")[:, _HEAD_PERM])
            m[f"L{l}_qb"] = _bias_cols(g("q_b")[_HEAD_PERM] / 8.0)
            m[f"L{l}_kw"] = g("k_w")
            m[f"L{l}_kb"] = g("k_b").reshape(P, 1)
            m[f"L{l}_vw"] = g("v_w")
            m[f"L{l}_vb"] = g("v_b").reshape(P, 1)
            m[f"L{l}_ow"] = g("o_w")
            m[f"L{l}_ob"] = _bias_cols(g("o_b"))
            m[f"L{l}_sink"] = g("sink").reshape(1, 1)
            if "w1" in bp:
                m[f"L{l}_w1"] = g("w1")
                m[f"L{l}_b1"] = _bias_cols(g("b1"))
                m[f"L{l}_w2"] = g("w2")
                m[f"L{l}_b2"] = _bias_cols(g("b2"))
            else:
                m[f"L{l}_rw"] = g("router_w") / 0.1
                m[f"L{l}_rb"] = (g("router_b") / 0.1).reshape(1, E)
                e0 = c * EPC
                m[f"L{l}_W1"] = np.ascontiguousarray(
                    g("W1")[e0:e0 + EPC].reshape(EPC * DIM, EH))
                m[f"L{l}_B1"] = np.concatenate(
                    [_bias_cols(g("b1e")[e0 + j]) for j in range(EPC)], axis=1)
                m[f"L{l}_W2"] = np.ascontiguousarray(
                    g("W2")[e0:e0 + EPC].reshape(EPC * EH, DIM))
                m[f"L{l}_B2"] = np.concatenate(
                    [_bias_cols(g("b2e")[e0 + j]) for j in range(EPC)], axis=1)
        in_maps.append(m)
    return in_maps


_NC_CACHE = {}


def get_nc():
    if "nc" not in _NC_CACHE:
        _NC_CACHE["nc"] = build_nc()
    return _NC_CACHE["nc"]


def run(input_ids, params, trace=False, **kw):
    nc = get_nc()
    in_maps = prepare_in_maps(input_ids, params)
    res = bass_utils.run_bass_kernel_spmd(
        nc, in_maps, core_ids=list(range(NCORES)), trace=trace, **kw)
    logits = np.concatenate([res.results[c]["logits"] for c in range(NCORES)],
                            axis=1)
    logits = logits.reshape(B, S, V)
    aux = np.float32(res.results[0]["aux"].reshape(()))
    return (logits, aux), res


def kernel(input_ids, params):
    out, _ = run(input_ids, params)
    return out
